# revision 13
# baseline (speedup 1.0000x reference)
"""GAT layer (N=8192, IN_F=512, OUT_F=64) on 8 Trainium2 NeuronCores.

Math: Wh = h @ W.T; e_ij = leaky_relu(s1_i + s2_j); att = softmax(e, axis=1);
out = att @ Wh, where s1 = Wh@a1, s2 = Wh@a2.

Key identity: with t = s1_i + s2_j,
  exp(leaky_relu(t)) = exp(s1_i)exp(s2_j)            if t >= 0
                       exp(a*s1_i)exp(a*s2_j)        if t <  0
so with p=exp(s1), q=exp(a*s1), u=exp(s2), v=exp(a*s2), M_ij = [t_ij>=0]:
  num_i = p_i * sum_j M_ij u_j Wh_j  +  q_i * (sum_j v_j Wh_j - sum_j M_ij v_j Wh_j)
  den_i = same with Wh_j -> 1
The only N^2 work is the 0/1 mask M (one dual-op DVE tensor_scalar per
j-chunk, f16 in/out for 4x mode) and matmuls against it.

Hot-loop orientation: the mask block [128j, 128i] is the PE *stationary*
operand; the moving operand is whuv = [u_j*[Wh_j|1] | v_j*[Wh_j|1]] (130
cols, f16). Each mask element is loaded once (FWL) instead of streamed
twice, halving PE column traffic vs the mask-as-rhs form. PSUM accumulates
directly in [i, f] layout (8 accumulators of [128, 130]), so no output
transposes; sum_j v_j Wh_j is folded in with a rank-1 fixup matmul.

Sharding: rows split across 8 cores; each core computes its Wh shard and
AllGathers it in 8 per-i-chunk pieces (overlapped with phase A), plus one
small gather of s=(s1,s2).
"""

import numpy as np

N, IN_F, OUT_F = 8192, 512, 64
ALPHA = 0.2
NCORES = 8
RPC = N // NCORES        # rows per core = 1024
NJC = N // 128           # 64 j-chunks over all rows
NIC = RPC // 128         # 8 i-chunks per core
NKC = IN_F // 128        # 4 k-chunks
F1 = OUT_F + 1           # 65: Wh columns + ones column for the denominator
FUV = 2 * F1             # 130: [u-scaled | v-scaled] moving operand

_CACHE = {}


def _build_kernel(unroll=1, sim_collectives=False, loop_reps=0, probe=0):
    return _build_kernel_impl(unroll, sim_collectives, loop_reps)


def _build_kernel_impl(unroll=1, sim_collectives=False, loop_reps=0):
    import concourse.bass as bass
    import concourse.bacc as bacc
    import concourse.tile as tile
    from concourse import mybir
    from concourse.masks import make_identity

    f32 = mybir.dt.float32
    f16 = mybir.dt.float16
    Alu = mybir.AluOpType
    Act = mybir.ActivationFunctionType

    nc = bacc.Bacc("TRN2", target_bir_lowering=False, debug=False,
                   num_devices=1 if sim_collectives else NCORES)
    h_d = nc.dram_tensor("h_shard", [RPC, IN_F], f32, kind="ExternalInput").ap()
    w_d = nc.dram_tensor("w_in", [OUT_F, IN_F], f32, kind="ExternalInput").ap()
    a_d = nc.dram_tensor("a_in", [2 * OUT_F, 1], f32, kind="ExternalInput").ap()
    out_d = nc.dram_tensor("out_shard", [RPC, OUT_F], f32,
                           kind="ExternalOutput").ap()

    with tile.TileContext(nc) as tc:
        with tc.tile_pool(name="dram", bufs=1, space="DRAM") as dram, \
             tc.tile_pool(name="singles", bufs=1) as singles:
            ident = singles.tile([128, 128], f32)
            make_identity(nc, ident)
            ones_col16 = singles.tile([128, 1], f16, name="ones_col16")
            nc.vector.memset(ones_col16, 1.0)
            ones_row16 = singles.tile([1, 128], f16, name="ones_row16")
            nc.vector.memset(ones_row16, 1.0)
            neg_row = singles.tile([1, 128], f32, name="neg_row")
            nc.vector.memset(neg_row, -1.0)

            if loop_reps > 0:
                _hints = (mybir.EngineType.PE, mybir.EngineType.DVE,
                          mybir.EngineType.Activation, mybir.EngineType.SP,
                          mybir.EngineType.Pool)
                with tc.For_i(0, loop_reps, 1, hint_engines=_hints):
                    _body(nc, tc, tile, bass, mybir, dram, singles, ident,
                          ones_col16, ones_row16, neg_row,
                          h_d, w_d, a_d, out_d, f32, f16, Alu, Act, 0,
                          sim_collectives)
            else:
                for _rep in range(unroll):
                    _body(nc, tc, tile, bass, mybir, dram, singles, ident,
                          ones_col16, ones_row16, neg_row,
                          h_d, w_d, a_d, out_d, f32, f16, Alu, Act, _rep,
                          sim_collectives)

    nc.compile()
    return nc


def _body(nc, tc, tile, bass, mybir, dram, singles, ident,
          ones_col16, ones_row16, neg_row,
          h_d, w_d, a_d, out_d, f32, f16, Alu, Act, rep,
          sim_collectives=False):
    # ---------------- Phase A: Wh for own rows; s1/s2 for own rows -------
    wh_own_dram = dram.tile([RPC, OUT_F], f32, name=f"wh_own_{rep}")
    _aspace = "Local" if sim_collectives else "Shared"
    # 8 chunked gather outputs: whg[k] holds rows [g*RPC + k*128, +128) of
    # the full Wh for every core g, laid out as [g*128 + p, f].
    whg_dram = [dram.tile([NCORES * 128, OUT_F], f32, addr_space=_aspace,
                          name=f"whg{k}_{rep}") for k in range(NIC)]
    # s gathered in two halves (cols 0:512 after ic3, 512:1024 after ic7)
    s_half_dram = [dram.tile([2, 512], f32, name=f"s_h{x}_{rep}")
                   for x in range(2)]
    s_full_dram = [dram.tile([2 * NCORES, 512], f32, addr_space=_aspace,
                             name=f"s_full{x}_{rep}") for x in range(2)]

    with tc.tile_pool(name="pha_sb", bufs=2) as pa, \
         tc.tile_pool(name="pha_ps", bufs=1, space="PSUM") as pap:
        w_sb = pa.tile([OUT_F, IN_F], f32, bufs=1)
        nc.sync.dma_start(out=w_sb, in_=w_d)
        # a as lhsT [64, 2]: col0 = a1, col1 = a2
        a_mat = pa.tile([OUT_F, 2], f32, bufs=1)
        nc.sync.dma_start(
            out=a_mat,
            in_=bass.AP(tensor=a_d.tensor, offset=0,
                        ap=[[1, OUT_F], [OUT_F, 2]]))

        # W.T tiles [k 128, f 64] via PE transpose, all 4 in one psum bank
        wt_all = pa.tile([128, NKC, OUT_F], f32, bufs=1)
        wt_ps = pap.tile([128, NKC, OUT_F], f32, bufs=1, tag="misc")
        for kc in range(NKC):
            nc.tensor.transpose(wt_ps[:, kc, :],
                                w_sb[:, kc * 128:(kc + 1) * 128],
                                ident[:OUT_F, :OUT_F])
        nc.scalar.copy(out=wt_all, in_=wt_ps)

        whT_all = pa.tile([OUT_F, RPC], f32, bufs=1)
        s_own_sb = singles.tile([2, RPC], f32, name=f"s_own_sb_{rep}")
        for ic in range(NIC):
            h_tile = pa.tile([128, IN_F], f32, bufs=3)
            nc.sync.dma_start(out=h_tile,
                              in_=h_d[ic * 128:(ic + 1) * 128, :])
            # transpose all 4 k-chunks into one [128, 4, 128] psum bank
            ht_ps = pap.tile([128, NKC, 128], f32, bufs=2)
            for kc in range(NKC):
                nc.tensor.transpose(ht_ps[:, kc, :],
                                    h_tile[:, kc * 128:(kc + 1) * 128],
                                    ident)
            ht_sb = pa.tile([128, NKC, 128], f32, bufs=3)
            nc.scalar.copy(out=ht_sb[:, 0:2, :], in_=ht_ps[:, 0:2, :])
            nc.vector.tensor_copy(out=ht_sb[:, 2:4, :], in_=ht_ps[:, 2:4, :])
            wh_ps = pap.tile([128, OUT_F], f32, bufs=2)
            for kc in range(NKC):
                nc.tensor.matmul(wh_ps, lhsT=ht_sb[:, kc, :],
                                 rhs=wt_all[:, kc, :],
                                 start=(kc == 0), stop=(kc == NKC - 1))
            wh_sb = pa.tile([128, OUT_F], f32, bufs=2)
            nc.scalar.copy(out=wh_sb, in_=wh_ps)
            nc.sync.dma_start(out=wh_own_dram[ic * 128:(ic + 1) * 128, :],
                              in_=wh_sb)
            # gather this i-chunk's Wh rows from all cores right away
            if sim_collectives:
                nc.gpsimd.dma_start(
                    out=bass.AP(tensor=whg_dram[ic].tensor, offset=0,
                                ap=[[128 * OUT_F, NCORES], [1, 128 * OUT_F]]),
                    in_=bass.AP(tensor=wh_own_dram.tensor,
                                offset=ic * 128 * OUT_F,
                                ap=[[0, NCORES], [1, 128 * OUT_F]]))
            else:
                nc.gpsimd.collective_compute(
                    "AllGather", mybir.AluOpType.bypass,
                    replica_groups=[list(range(NCORES))],
                    ins=[wh_own_dram[ic * 128:(ic + 1) * 128, :].opt()],
                    outs=[whg_dram[ic].opt()])
            # Wh.T slice via PE transpose of wh_sb
            whT_ps = pap.tile([OUT_F, 128], f32, bufs=2)
            nc.tensor.transpose(whT_ps, wh_sb, ident)
            nc.vector.tensor_copy(out=whT_all[:, ic * 128:(ic + 1) * 128],
                                  in_=whT_ps)
            # s for this half as soon as its 4 ics are transposed; gather
            # the half right away so phase C can start with k<4 early
            if ic == 3 or ic == NIC - 1:
                half = 0 if ic == 3 else 1
                s_ps = pap.tile([2, 512], f32, bufs=1, tag="misc")
                nc.tensor.matmul(s_ps, lhsT=a_mat,
                                 rhs=whT_all[:, half * 512:(half + 1) * 512],
                                 start=True, stop=True)
                nc.scalar.copy(out=s_own_sb[:, half * 512:(half + 1) * 512],
                               in_=s_ps)
                nc.sync.dma_start(out=s_half_dram[half],
                                  in_=s_own_sb[:, half * 512:(half + 1) * 512])
                if sim_collectives:
                    nc.gpsimd.dma_start(
                        out=bass.AP(tensor=s_full_dram[half].tensor, offset=0,
                                    ap=[[2 * 512, NCORES], [1, 2 * 512]]),
                        in_=bass.AP(tensor=s_half_dram[half].tensor, offset=0,
                                    ap=[[0, NCORES], [1, 2 * 512]]))
                else:
                    nc.gpsimd.collective_compute(
                        "AllGather", mybir.AluOpType.bypass,
                        replica_groups=[list(range(NCORES))],
                        ins=[s_half_dram[half].opt()],
                        outs=[s_full_dram[half].opt()])

    # ------------- Phase C: prep small tensors (needs s_full halves) -----
    if True:
        sc = singles
        # s1 of own rows broadcast across partitions as f16 [128, RPC],
        # via PE rank-1 matmul (neg_row x s1_row, negated on copy-out).
        s1b = sc.tile([128, RPC], f16, name=f"s1b_{rep}")
        s_cols = sc.tile([128, NIC, 2 * NCORES], f32, name=f"s_cols_{rep}")
        u_cols = sc.tile([128, NIC, 2 * NCORES], f32, name=f"u_cols_{rep}")
        v_cols = sc.tile([128, NIC, 2 * NCORES], f32, name=f"v_cols_{rep}")
        with tc.tile_pool(name="phc_ps", bufs=1, space="PSUM") as pcp:
            tr_ps = pcp.tile([128, NIC, 2 * NCORES], f32)
            for half in range(2):
                # per-j column layouts for this half's 4 k-chunks:
                # s_cols[p, k, 2g+1] = s2 of global row (g*RPC + k*128 + p)
                s_half_sb = sc.tile([2 * NCORES, 512], f32,
                                    name=f"s_half_sb{half}_{rep}")
                nc.sync.dma_start(out=s_half_sb, in_=s_full_dram[half])
                ks = range(4 * half, 4 * half + 4)
                for k in ks:
                    nc.tensor.transpose(
                        tr_ps[:, k, :],
                        s_half_sb[:, (k - 4 * half) * 128:
                                  (k - 4 * half + 1) * 128],
                        ident[:2 * NCORES, :2 * NCORES])
                ksl = slice(4 * half, 4 * half + 4)
                nc.vector.tensor_copy(out=s_cols[:, ksl, :],
                                      in_=tr_ps[:, ksl, :])
                nc.scalar.activation(out=u_cols[:, ksl, :],
                                     in_=s_cols[:, ksl, :], func=Act.Exp)
                nc.scalar.activation(out=v_cols[:, ksl, :],
                                     in_=s_cols[:, ksl, :], func=Act.Exp,
                                     scale=ALPHA)

            s1b_ps0 = pcp.tile([128, 512], f32)
            s1b_ps1 = pcp.tile([128, 512], f32)
            nc.tensor.matmul(s1b_ps0, lhsT=neg_row, rhs=s_own_sb[0:1, 0:512],
                             start=True, stop=True)
            nc.tensor.matmul(s1b_ps1, lhsT=neg_row, rhs=s_own_sb[0:1, 512:1024],
                             start=True, stop=True)
            nc.scalar.activation(out=s1b[:, 0:512], in_=s1b_ps0,
                                 func=Act.Copy, scale=-1.0)
            nc.vector.tensor_scalar(out=s1b[:, 512:1024], in0=s1b_ps1,
                                    scalar1=-1.0, scalar2=None, op0=Alu.mult)

            # own s1 in per-partition columns: s1_cols[p, k] = s1[k*128+p]
            tr2_ps = pcp.tile([128, NIC, 2], f32)
            for k in range(NIC):
                nc.tensor.transpose(tr2_ps[:, k, :],
                                    s_own_sb[:, k * 128:(k + 1) * 128],
                                    ident[:2, :2])
            s1_cols = sc.tile([128, NIC, 2], f32, name=f"s1_cols_{rep}")
            nc.scalar.copy(out=s1_cols, in_=tr2_ps)

        p_cols = sc.tile([128, NIC, 2], f32, name=f"p_cols_{rep}")
        nc.scalar.activation(out=p_cols, in_=s1_cols, func=Act.Exp)
        q_cols = sc.tile([128, NIC, 2], f32, name=f"q_cols_{rep}")
        nc.scalar.activation(out=q_cols, in_=s1_cols, func=Act.Exp, scale=ALPHA)

    # ---------------- Phase D: masks as PE weights, accumulate [i, f] ----
    sv_sb = singles.tile([1, FUV], f32, name=f"sv_sb_{rep}")
    with tc.tile_pool(name="phd_sb", bufs=3) as pd, \
         tc.tile_pool(name="phd_uv", bufs=6) as pw, \
         tc.tile_pool(name="phd_mask", bufs=8) as pdm, \
         tc.tile_pool(name="phd_ps", bufs=1, space="PSUM") as pdp, \
         tc.tile_pool(name="phe_sb", bufs=3) as pe:
        # 4 bank-aligned psum tiles, 2 accumulators each: [i, f] layout
        pair_ps = [pdp.tile([128, 2, 256], f32, name=f"pair{x}_{rep}")
                   for x in range(4)]
        sv_ps = pdp.tile([1, FUV], f32, name=f"svp_{rep}")

        jidx = 0
        for k in range(NIC):
            for hlf in range(2):
                whc4 = pd.tile([128, 4, F1], f32)
                nc.vector.memset(whc4[:, :, OUT_F:F1], 1.0)
                nc.sync.dma_start(
                    out=whc4[:, :, 0:OUT_F],
                    in_=bass.AP(tensor=whg_dram[k].tensor,
                                offset=hlf * 4 * 128 * OUT_F,
                                ap=[[OUT_F, 128], [128 * OUT_F, 4],
                                    [1, OUT_F]]))
                for g4 in range(4):
                    g = hlf * 4 + g4
                    jc = g * NIC + k
                    mask = pdm.tile([128, RPC], f16)
                    nc.vector.tensor_scalar(
                        out=mask, in0=s1b,
                        scalar1=s_cols[:, k, 2 * g + 1:2 * g + 2],
                        scalar2=0.0, op0=Alu.add, op1=Alu.is_ge)
                    whuv = pw.tile([128, FUV], f16)
                    nc.scalar.activation(out=whuv[:, 0:F1],
                                         in_=whc4[:, g4, :], func=Act.Copy,
                                         scale=u_cols[:, k, 2 * g + 1:2 * g + 2])
                    nc.vector.tensor_scalar(
                        out=whuv[:, F1:FUV], in0=whc4[:, g4, :],
                        scalar1=v_cols[:, k, 2 * g + 1:2 * g + 2],
                        scalar2=None, op0=Alu.mult)
                    st = (jidx == 0)
                    sp = (jidx == NJC - 1)
                    for ic in range(NIC):
                        # one accumulation group per PSUM bank: only the
                        # bank's first MM starts it, only its last stops it
                        nc.tensor.matmul(
                            pair_ps[ic // 2][:, ic % 2, 0:FUV],
                            lhsT=mask[:, ic * 128:(ic + 1) * 128],
                            rhs=whuv, start=(st and ic % 2 == 0),
                            stop=(sp and ic % 2 == 1))
                    nc.tensor.matmul(sv_ps, lhsT=ones_col16, rhs=whuv,
                                     start=st, stop=sp)
                    jidx += 1

        # ---------------- Phase E: combine, divide, store ----------------
        # out[i, f] = p_i*Du[i, f] - q_i*(Dv[i, f] - Sv[f]); den = col 64.
        # Fold -Sv into the v-half of each accumulator with a rank-1 matmul.
        nc.scalar.copy(out=sv_sb, in_=sv_ps)
        out_sb = singles.tile([128, NIC, OUT_F], f32, name=f"out_sb_{rep}")
        for ic in range(NIC):
            pv = pair_ps[ic // 2][:, ic % 2, :]
            nc.tensor.matmul(pv[:, F1:FUV], lhsT=neg_row,
                             rhs=sv_sb[:, F1:FUV], start=False, stop=True,
                             skip_group_check=True)
            r1 = pe.tile([128, F1], f32)
            nc.scalar.activation(out=r1, in_=pv[:, 0:F1], func=Act.Copy,
                                 scale=p_cols[:, ic, 0:1])
            r2 = pe.tile([128, F1], f32)
            nc.scalar.activation(out=r2, in_=pv[:, F1:FUV], func=Act.Copy,
                                 scale=q_cols[:, ic, 0:1])
            r4 = pe.tile([128, F1], f32)
            nc.gpsimd.tensor_tensor(out=r4, in0=r1, in1=r2, op=Alu.subtract)
            rec = pe.tile([128, 1], f32)
            nc.vector.reciprocal(out=rec, in_=r4[:, OUT_F:F1])
            nc.vector.tensor_scalar(out=out_sb[:, ic, :], in0=r4[:, 0:OUT_F],
                                    scalar1=rec, scalar2=None, op0=Alu.mult)
        # single batched store: out_d[ic*128 + p, f] = out_sb[p, ic, f]
        nc.sync.dma_start(
            out=bass.AP(tensor=out_d.tensor, offset=0,
                        ap=[[OUT_F, 128], [128 * OUT_F, NIC], [1, OUT_F]]),
            in_=out_sb)


def _get_nc(unroll=1):
    key = ("nc", unroll)
    if key not in _CACHE:
        _CACHE[key] = _build_kernel(unroll)
    return _CACHE[key]


def kernel(h, adj, W, a, _unroll=1, _return_raw=False):
    from concourse.bass_utils import run_bass_kernel_spmd

    nc = _get_nc(_unroll)
    h = np.ascontiguousarray(np.asarray(h, dtype=np.float32))
    W = np.ascontiguousarray(np.asarray(W, dtype=np.float32))
    a = np.ascontiguousarray(np.asarray(a, dtype=np.float32))
    in_maps = [
        {"h_shard": h[c * RPC:(c + 1) * RPC], "w_in": W, "a_in": a}
        for c in range(NCORES)
    ]
    res = run_bass_kernel_spmd(nc, in_maps, list(range(NCORES)))
    out = np.concatenate([res.results[c]["out_shard"] for c in range(NCORES)],
                         axis=0)
    if _return_raw:
        return out, res
    return out
